# revision 48
# baseline (speedup 1.0000x reference)
"""ArcFace logits on 8 Trainium2 NeuronCores (Bass, raw engine streams).

out[n, c] = S * cos(theta_nc + M * [c == labels[n]]),  cos from L2-normalized
embeddings [1024, 512] x weight [100000, 512].

Model-parallel over the class dim (partial-FC): classes are padded/permuted
on the host so every core gets 12800 columns and its 128 label hits land on
the diagonal of output tile (chunk 0, row-block 0).  The compiled graph is
identical on all 8 cores and label-independent.

Final schedule (~192.5us/core vs the 222.4us previous best; PE array busy
~171us = the bf16 roofline for 8 x 25-chunk x 4-k matmuls at 2.4GHz):
  - BOTH L2 normalizations (and the S scale) are folded into the host-packed
    bf16 inputs, so the device does nothing but matmul + PSUM->f16 eviction
    + DMA.  No on-device norms: the whole ssq/rsqw/rsqe machinery (PE ssq
    matmuls, DVE squares/adds, ACT Ln/Exp chains) of the old design is gone,
    making the eviction engines 3x underloaded instead of co-critical.
  - zero class padding: CS=12500 exactly (24 full 512-wide chunks + one
    212-wide), saving ~2.3% of PE rows; the handful of duplicate-label
    classes are recomputed exactly on the host (2 entries per duplicate).
  - PSUM = 4 pair-banks of [128,1024]: unit u writes bank u%4, giving the
    eviction stream 3 units of slack before it can ever gate the PE.
  - evictions are pure f32->f16 copies SPLIT across DVE (even units) and
    ACT (odd units), each ~57us busy vs the PE's ~171us.
  - per-unit immediate out-DMA (256KB) right after each eviction.  DMA
    ISSUE bandwidth is the scarce resource, not DMA transfer bandwidth:
    ACT issues its own out-DMAs in-order post-eviction, the otherwise-idle
    SP sequencer issues DVE's (DVE cannot drive HWDGE), and gpsimd (slow
    SWDGE, ~6us issue-to-data) only gets latency-insensitive work.
  - the start is input-DMA-bound: eT is loaded as 8 per-row-block pieces and
    wt0 as 4 per-k pieces (each on its own semaphore - completion counts of
    equal DMAs on a shared semaphore can reorder across queues, which was a
    real intermittent race), issued from SP+ACT in consumption order, so the
    first matmul starts at ~10.6us and the PE never stalls >0.6us after it.
  - margin applied on the diagonal of tile (pair0, rb0): DVE gathers the
    diag, ACT does the trig, DVE rewrites the diagonal; only that one
    unit's out-DMA (issued from gpsimd) gates on the fix.
"""

import math

import numpy as np
import ml_dtypes

import concourse.bass as bass
import concourse.mybir as mybir
from concourse.bass_utils import run_bass_kernel_spmd

AF = mybir.ActivationFunctionType
OP = mybir.AluOpType
F32 = mybir.dt.float32
F16 = mybir.dt.float16
BF16 = mybir.dt.bfloat16

S = 30.0
MARGIN = 0.5
N, D, C = 1024, 512, 100000

NCORES = 8
F = 512               # matmul free dim / class chunk width
F24 = 212             # trimmed width of the leading single chunk
CS = 24 * F + F24     # 12500 classes per core: 8 * 12500 = 100000, no padding
NCHUNK = 25
KD = D // 128         # 4 contraction sub-tiles
NB = N // 128         # 8 row blocks
NWT = 8               # wt chunk buffers
NPB = 8               # pair out-buffer rotation per engine
NPAIR = (NCHUNK - 1) // 2  # 12 pairs after the leading single chunk

COSM = float(math.cos(MARGIN))
SINM = float(math.sin(MARGIN))

# chunk processing order: odd chunk 24 first, then pairs (0,1),(2,3),...
SEQ = [24] + list(range(24))


def _evc(u):
    """('d'|'a', count) such that `sem >= count` implies unit u is evicted.
    DVE evicts even units in order, ACT odd units."""
    if u < 8:
        return ("d" if u % 2 == 0 else "a", u // 2 + 1)
    return ("d" if u % 2 == 0 else "a", 4 + (u - 8) // 2 + 1)


def _munits(oo):
    """s_mmu value once all units of seq-chunk oo are complete."""
    if oo <= 0:
        return NB if oo == 0 else 0
    return NB + NB * ((oo - 1) // 2 + 1)


def build_graph():
    nc = bass.Bass(target_bir_lowering=False)

    eT_ext = nc.declare_dram_parameter("eT", [128, NB * KD * 128], BF16, isOutput=False)
    w_ext = nc.declare_dram_parameter("w", [128, 24 * KD * F + KD * F24], BF16,
                                      isOutput=False)
    ident_ext = nc.declare_dram_parameter("ident", [128, 128], F32, isOutput=False)
    out_ext = nc.declare_dram_parameter("out", [N, CS], F16, isOutput=True)

    import contextlib

    ctx = contextlib.ExitStack()
    sb = lambda name, shape, dt=F32: ctx.enter_context(nc.sbuf_tensor(name, shape, dt))
    sem = lambda name: ctx.enter_context(nc.semaphore(name))

    with ctx:
        # --- SBUF ---
        eT_sb = sb("eT_sb", [128, NB * KD * 128], BF16)
        wt = [sb(f"wt{b}", [128, KD * F], BF16) for b in range(NWT)]
        souts = [sb(f"souts{t}", [128, F24], F16) for t in range(NB)]
        poutd = [sb(f"poutd{b}", [128, 2 * F], F16) for b in range(NPB)]
        pouta = [sb(f"pouta{b}", [128, 2 * F], F16) for b in range(NPB)]
        ident_sb = sb("ident_sb", [128, 128])
        diag_tmp = sb("diag_tmp", [128, 128])
        vdiag = sb("vdiag", [128, 1])
        sqv = sb("sqv", [128, 1])
        lnu = sb("lnu", [128, 1])
        s3v = sb("s3v", [128, 1])
        t1v = sb("t1v", [128, 1])
        fixp = sb("fixp", [128, 1])
        deltap = sb("deltap", [128, 1])
        s2_b = sb("s2_b", [128, 1])

        # --- PSUM: 4 pair-banks [128,1024] = 16KB/partition ---
        ps_pair = [
            ctx.enter_context(nc.psum_tensor(f"ps_pair{b}", [128, 2 * F], F32))
            for b in range(4)
        ]

        # --- semaphores ---
        s_ident = sem("s_ident")
        s_eTt = [sem(f"s_eTt{t}") for t in range(NB)]
        s_ms = sem("s_ms")
        s_do8 = sem("s_do8")
        # one sem per wt0 k-piece: completions of equal-priority DMAs can
        # reorder across queues, so partial counts of a shared sem are unsafe
        s_w24k = [sem(f"s_w24k{k}") for k in range(KD)]
        s_wt = [sem(f"s_wt{b}") for b in range(NWT)]
        s_mmu = sem("s_mmu")       # PE unit done (104 total)
        s_evd = sem("s_evd")       # DVE evictions done
        s_eva = sem("s_eva")       # ACT evictions done
        s_dod = sem("s_dod")       # DVE-side out-DMA completions (x16)
        s_doa = sem("s_doa")       # ACT-side out-DMA completions (x16)
        s_do24 = sem("s_do24")     # chunk-24 out-DMA completions (x16)
        s_vg = sem("s_vg")
        s_sfix = sem("s_sfix")
        s_vfix = sem("s_vfix")

        ev_sem = {"d": s_evd, "a": s_eva}

        with nc.Block() as block:

            @block.gpsimd
            def _(g):
                def wt_dma(o):
                    c = SEQ[o]
                    g.dma_start(
                        out=wt[o % NWT][:],
                        in_=w_ext[:, c * KD * F:(c + 1) * KD * F],
                    ).then_inc(s_wt[o % NWT], 16)

                # gpsimd DMAs go through SWDGE (~6us issue-to-data): only
                # latency-insensitive loads live here; everything the first
                # 20us needs is issued from SP/ACT (HWDGE)
                g.dma_start(out=ident_sb[:], in_=ident_ext[:]).then_inc(s_ident, 16)
                g.memset(s2_b[:], float(S * S)).then_inc(s_ms, 1)
                # unit 8 (pair0, rb0) holds the label diagonal: its out-DMA
                # is the only one gated on the margin fix, issued from here
                # (the evicting engines issue every other out-DMA themselves)
                g.wait_ge(s_vfix, 1)
                g.dma_start(
                    out=out_ext[0:128, 0:2 * F], in_=poutd[0][:],
                ).then_inc(s_do8, 16)
                # wt5/wt6 deferred past the critical eT/wt0/wt1/wt2 window:
                # the margin fix (~pair0 mid) is long before pair 2 needs them
                wt_dma(5)
                wt_dma(6)
                for p in range(NPAIR):
                    for o in (2 * p + 7, 2 * p + 8):
                        if o <= NCHUNK - 1:
                            oo = o - NWT
                            if oo >= 0:
                                g.wait_ge(s_mmu, _munits(oo))
                            wt_dma(o)
                g.wait_ge(s_do24, 16 * NB)
                g.wait_ge(s_do8, 16)
                g.wait_ge(s_dod, 16 * (4 * NPAIR - 1))
                g.wait_ge(s_doa, 16 * 4 * NPAIR)

            @block.scalar
            def _(s):
                # dummy op: pulls the ACT table load off the critical path
                s.activation(sqv[:], vdiag[:], AF.Square)
                # ACT issues the wt0 k-pieces (tiny, needed first), the odd
                # eT pieces, then the pair-0 chunks, in consumption order
                W24 = 24 * KD * F
                for k in range(KD):
                    s.dma_start(
                        out=wt[0][:, k * F24:(k + 1) * F24],
                        in_=w_ext[:, W24 + k * F24:W24 + (k + 1) * F24],
                    ).then_inc(s_w24k[k], 16)
                for t_ in (1, 3, 5, 7):
                    s.dma_start(
                        out=eT_sb[:, t_ * KD * 128:(t_ + 1) * KD * 128],
                        in_=eT_ext[:, t_ * KD * 128:(t_ + 1) * KD * 128],
                    ).then_inc(s_eTt[t_], 16)
                s.dma_start(
                    out=wt[1][:],
                    in_=w_ext[:, SEQ[1] * KD * F:(SEQ[1] + 1) * KD * F],
                ).then_inc(s_wt[1], 16)
                s.dma_start(
                    out=wt[2][:],
                    in_=w_ext[:, SEQ[2] * KD * F:(SEQ[2] + 1) * KD * F],
                ).then_inc(s_wt[2], 16)
                # chunk-24 odd tiles; the out-DMA is issued right here on the
                # scalar sequencer (in-order after the eviction, HWDGE)
                for tt in (1, 3, 5, 7):
                    s.wait_ge(s_mmu, tt + 1)
                    s.activation(
                        souts[tt][:],
                        ps_pair[tt % 4][:, (tt // 4) * F:(tt // 4) * F + F24],
                        AF.Copy,
                    ).then_inc(s_eva, 1)
                    s.dma_start(
                        out=out_ext[tt * 128:(tt + 1) * 128, 24 * F:24 * F + F24],
                        in_=souts[tt][:],
                    ).then_inc(s_do24, 16)
                    if tt == 5:
                        s.dma_start(
                            out=wt[3][:],
                            in_=w_ext[:, SEQ[3] * KD * F:(SEQ[3] + 1) * KD * F],
                        ).then_inc(s_wt[3], 16)
                    if tt == 7:
                        s.dma_start(
                            out=wt[4][:],
                            in_=w_ext[:, SEQ[4] * KD * F:(SEQ[4] + 1) * KD * F],
                        ).then_inc(s_wt[4], 16)
                # odd pair units
                for j in range(4 * NPAIR):
                    u = 9 + 2 * j
                    p, nb = (u - 8) // 8, (u - 8) % 8
                    if j >= NPB:
                        s.wait_ge(s_doa, 16 * (j - NPB + 1))
                    s.wait_ge(s_mmu, u + 1)
                    s.activation(pouta[j % NPB][:], ps_pair[u % 4][:],
                                 AF.Copy).then_inc(s_eva, 1)
                    s.dma_start(
                        out=out_ext[nb * 128:(nb + 1) * 128,
                                    2 * p * F:(2 * p + 2) * F],
                        in_=pouta[j % NPB][:],
                    ).then_inc(s_doa, 16)
                    if j == 0:
                        # margin trig on the gathered diagonal [128,1]
                        s.wait_ge(s_vg, 1)
                        s.wait_ge(s_ms, 1)
                        s.activation(sqv[:], vdiag[:], AF.Square)
                        s.drain()
                        s.activation(lnu[:], sqv[:], AF.Ln, scale=-1.0, bias=s2_b[:])
                        s.drain()
                        s.activation(s3v[:], lnu[:], AF.Exp, scale=0.5)
                        s.activation(t1v[:], vdiag[:], AF.Copy, scale=COSM)
                        s.drain().then_inc(s_sfix, 1)

            @block.tensor
            def _(t):
                # chunk 24 (trimmed to F24): tile tt -> bank tt%4, half tt//4
                for tt in range(NB):
                    t.wait_ge(s_eTt[tt], 16)
                    bank, half = tt % 4, tt // 4
                    for k in range(KD):
                        if tt == 0:
                            t.wait_ge(s_w24k[k], 16)
                        mm = t.matmul(
                            ps_pair[bank][:, half * F:half * F + F24],
                            lhsT=eT_sb[:, tt * 512 + k * 128:tt * 512 + (k + 1) * 128],
                            rhs=wt[0][:, k * F24:(k + 1) * F24],
                            start=(k == 0), stop=(k == KD - 1),
                            skip_group_check=True,
                        )
                    mm.then_inc(s_mmu, 1)

                def wt_thr(o):
                    # wt[0]'s initial fill is tracked on s_w24k instead, so
                    # s_wt[0] only counts its refills (o = 8, 16, 24)
                    if o % NWT == 0:
                        return 16 * (o // NWT)
                    return 16 * (o // NWT + 1)

                for p in range(NPAIR):
                    o0, o1 = 2 * p + 1, 2 * p + 2
                    t.wait_ge(s_wt[o0 % NWT], wt_thr(o0))
                    t.wait_ge(s_wt[o1 % NWT], wt_thr(o1))
                    for nb in range(NB):
                        u = NB + NB * p + nb
                        # bank u%4 free once its previous occupants evicted
                        if u < 12:
                            b = u - 8
                            sd, cnt = _evc(b)          # tile b (half 0)
                            _, cnt2 = _evc(b + 4)      # tile b+4 (half 1)
                            t.wait_ge(ev_sem[sd], max(cnt, cnt2))
                        else:
                            sd, cnt = _evc(u - 4)
                            t.wait_ge(ev_sem[sd], cnt)
                        for k in range(KD):
                            for j, ob in ((0, o0), (1, o1)):
                                mm = t.matmul(
                                    ps_pair[u % 4][:, j * F:(j + 1) * F],
                                    lhsT=eT_sb[:, nb * 512 + k * 128:
                                               nb * 512 + (k + 1) * 128],
                                    rhs=wt[ob % NWT][:, k * F:(k + 1) * F],
                                    start=(k == 0), stop=(k == KD - 1),
                                    skip_group_check=True,
                                )
                        mm.then_inc(s_mmu, 1)

            @block.sync
            def _(sy):
                # SP issues the even eT pieces, eT0 first so tile 0 can
                # start as early as possible
                for t_ in (0, 2, 4, 6):
                    sy.dma_start(
                        out=eT_sb[:, t_ * KD * 128:(t_ + 1) * KD * 128],
                        in_=eT_ext[:, t_ * KD * 128:(t_ + 1) * KD * 128],
                    ).then_inc(s_eTt[t_], 16)
                # DVE can't drive HWDGE; the otherwise-idle SP sequencer
                # issues the out-DMAs for DVE's evictions, paced by s_evd
                for tt in (0, 2, 4, 6):
                    sy.wait_ge(s_evd, tt // 2 + 1)
                    sy.dma_start(
                        out=out_ext[tt * 128:(tt + 1) * 128, 24 * F:24 * F + F24],
                        in_=souts[tt][:],
                    ).then_inc(s_do24, 16)
                # unit 8 (j==0) is DMAed from gpsimd after the margin fix
                for j in range(1, 4 * NPAIR):
                    u = 8 + 2 * j
                    p, nb = (u - 8) // 8, (u - 8) % 8
                    sy.wait_ge(s_evd, 4 + j + 1)
                    sy.dma_start(
                        out=out_ext[nb * 128:(nb + 1) * 128,
                                    2 * p * F:(2 * p + 2) * F],
                        in_=poutd[j % NPB][:],
                    ).then_inc(s_dod, 16)

            @block.vector
            def _(v):
                # chunk-24 even tiles
                for tt in (0, 2, 4, 6):
                    v.wait_ge(s_mmu, tt + 1)
                    v.tensor_scalar_mul(
                        souts[tt][:],
                        ps_pair[tt % 4][:, (tt // 4) * F:(tt // 4) * F + F24],
                        1.0,
                    ).then_inc(s_evd, 1)
                # even pair units
                for j in range(4 * NPAIR):
                    u = 8 + 2 * j
                    if j == NPB:
                        # poutd[0] was DMAed by gpsimd (vfix path), tracked
                        # on its own semaphore
                        v.wait_ge(s_do8, 16)
                    elif j > NPB:
                        v.wait_ge(s_dod, 16 * (j - NPB))
                    v.wait_ge(s_mmu, u + 1)
                    v.tensor_scalar_mul(poutd[j % NPB][:], ps_pair[u % 4][:],
                                        1.0).then_inc(s_evd, 1)
                    if j == 0:
                        # extract the label diagonal of tile (pair0, rb0)
                        v.wait_ge(s_ident, 16)
                        v.drain()
                        v.tensor_tensor(out=diag_tmp[:], in0=poutd[0][:, 0:128],
                                        in1=ident_sb[:], op=OP.mult)
                        v.drain()
                        v.tensor_reduce(vdiag[:], diag_tmp[:],
                                        mybir.AxisListType.X, OP.add)
                        v.drain().then_inc(s_vg, 1)
                    if j == 2:
                        # margin rewrite of the diagonal after the trig lands
                        v.wait_ge(s_sfix, 1)
                        v.scalar_tensor_tensor(fixp[:], s3v[:], -SINM, t1v[:],
                                               OP.mult, OP.add)
                        v.drain()
                        v.tensor_tensor(out=deltap[:], in0=fixp[:], in1=vdiag[:],
                                        op=OP.subtract)
                        v.drain()
                        v.scalar_tensor_tensor(poutd[0][:, 0:128], ident_sb[:],
                                               deltap[:], poutd[0][:, 0:128],
                                               OP.mult, OP.add)
                        v.drain().then_inc(s_vfix, 1)

    return nc


_GRAPH = None


def _get_graph():
    global _GRAPH
    if _GRAPH is None:
        _GRAPH = build_graph()
    return _GRAPH


def _host_prepare(embeddings, weight, labels):
    """Row/class permutations putting each core's labels on the (0,0) diagonal,
    with both L2 norms and the S scale folded in, packed into
    partition-contiguous bf16 layouts.

    CS * NCORES == C exactly, so every class gets exactly one column.  When a
    class is the label of two rows, the later row's diagonal slot is given a
    replacement (unlabeled) class instead; both affected entries of the output
    are recomputed exactly on the host afterwards (a handful per batch)."""
    labels = np.asarray(labels).astype(np.int64)
    e = np.asarray(embeddings, dtype=np.float32)
    w = np.asarray(weight, dtype=np.float32)

    # host-side normalization (matches torch F.normalize eps semantics)
    en = S * e / np.maximum(np.linalg.norm(e, axis=1, keepdims=True), 1e-12)
    wn = w / np.maximum(np.linalg.norm(w, axis=1, keepdims=True), 1e-12)

    seen = set()
    dups = []  # (row i, label) for duplicate label classes
    for i in range(N):
        l = int(labels[i])
        if l in seen:
            dups.append((i, l))
        else:
            seen.add(l)

    labeled = np.zeros(C, dtype=bool)
    labeled[labels] = True
    unlab = list(np.nonzero(~labeled)[0])

    colmaps = np.empty((NCORES, CS), dtype=np.int64)
    for i in range(N):
        colmaps[i // 128, i % 128] = labels[i]
    fixes = []  # (row i, label l, replacement r)
    for (i, l) in dups:
        r = int(unlab.pop())
        colmaps[i // 128, i % 128] = r
        fixes.append((i, l, r))
    colmaps[:, 128:] = np.asarray(unlab, dtype=np.int64).reshape(
        NCORES, CS - 128)

    e_bf = en.astype(ml_dtypes.bfloat16)
    wTfull = wn.T  # [512, 100000] view
    ident = np.eye(128, dtype=np.float32)
    in_maps = []
    row_perms = []
    for m in range(NCORES):
        wsel = np.ascontiguousarray(wTfull[:, colmaps[m]])
        w_bf = wsel.astype(ml_dtypes.bfloat16)
        # [D=(ko,p), c] -> [p, chunk, ko, f] for the 24 full chunks,
        # then [p, ko, f24] for the trimmed chunk
        wmain = np.ascontiguousarray(
            w_bf[:, :24 * F].reshape(KD, 128, 24, F).transpose(1, 2, 0, 3)
        ).reshape(128, 24 * KD * F)
        wtail = np.ascontiguousarray(
            w_bf[:, 24 * F:].reshape(KD, 128, F24).transpose(1, 0, 2)
        ).reshape(128, KD * F24)
        wprep = np.concatenate([wmain, wtail], axis=1)
        rows = np.concatenate([
            np.arange(m * 128, (m + 1) * 128),
            np.delete(np.arange(N), np.s_[m * 128:(m + 1) * 128]),
        ])
        row_perms.append(rows)
        e_perm = e_bf[rows]                      # [N, D]
        # [D=(k,dp), N=(nb,np)] -> [dp, nb, k, np]
        eTprep = np.ascontiguousarray(
            e_perm.T.reshape(KD, 128, NB, 128).transpose(1, 2, 0, 3)
        ).reshape(128, NB * KD * 128)
        in_maps.append({
            "eT": eTprep,
            "w": wprep,
            "ident": ident,
        })
    return in_maps, row_perms, colmaps, fixes, en, wn


def _assemble(results, row_perms, colmaps, fixes, en, wn):
    out = np.empty((N, C), dtype=np.float32)
    for m in range(NCORES):
        slab = results[m]["out"].astype(np.float32)
        unperm = np.empty_like(slab)
        unperm[row_perms[m]] = slab
        out[:, colmaps[m]] = unperm
    for (i, l, r) in fixes:
        # r's column got the margin on the device at row i: recompute exactly
        out[i, r] = float(en[i] @ wn[r])
        # row i's true label column is elsewhere, unmargined: apply margin
        cos = min(1.0, max(-1.0, out[i, l] / S))
        out[i, l] = S * math.cos(math.acos(cos) + MARGIN)
    return out


def kernel(embeddings, weight, labels, _trace=False):
    nc = _get_graph()
    in_maps, row_perms, colmaps, fixes, en, wn = _host_prepare(
        embeddings, weight, labels
    )
    res = run_bass_kernel_spmd(nc, in_maps, core_ids=list(range(NCORES)), trace=_trace)
    out = _assemble(res.results, row_perms, colmaps, fixes, en, wn)
    if _trace:
        return out, res
    return out


# revision 52
# speedup vs baseline: 1.0232x; 1.0232x over previous
"""ArcFace logits on 8 Trainium2 NeuronCores (Bass, raw engine streams).

out[n, c] = S * cos(theta_nc + M * [c == labels[n]]),  cos from L2-normalized
embeddings [1024, 512] x weight [100000, 512].

Model-parallel over the class dim (partial-FC): classes are padded/permuted
on the host so every core gets 12800 columns and its 128 label hits land on
the diagonal of output tile (chunk 0, row-block 0).  The compiled graph is
identical on all 8 cores and label-independent.

Final schedule (~192.5us/core vs the 222.4us previous best; PE array busy
~171us = the bf16 roofline for 8 x 25-chunk x 4-k matmuls at 2.4GHz):
  - BOTH L2 normalizations (and the S scale) are folded into the host-packed
    bf16 inputs, so the device does nothing but matmul + PSUM->f16 eviction
    + DMA.  No on-device norms: the whole ssq/rsqw/rsqe machinery (PE ssq
    matmuls, DVE squares/adds, ACT Ln/Exp chains) of the old design is gone,
    making the eviction engines 3x underloaded instead of co-critical.
  - zero class padding: CS=12500 exactly (24 full 512-wide chunks + one
    212-wide), saving ~2.3% of PE rows; the handful of duplicate-label
    classes are recomputed exactly on the host (2 entries per duplicate).
  - PSUM = 4 pair-banks of [128,1024]: unit u writes bank u%4, giving the
    eviction stream 3 units of slack before it can ever gate the PE.
  - evictions are pure f32->f16 copies SPLIT across DVE (even units) and
    ACT (odd units), each ~57us busy vs the PE's ~171us.
  - per-unit immediate out-DMA (256KB) right after each eviction.  DMA
    ISSUE bandwidth is the scarce resource, not DMA transfer bandwidth:
    ACT issues its own out-DMAs in-order post-eviction, the otherwise-idle
    SP sequencer issues DVE's (DVE cannot drive HWDGE), and gpsimd (slow
    SWDGE, ~6us issue-to-data) only gets latency-insensitive work.
  - the start is input-DMA-bound: eT is loaded as 8 per-row-block pieces and
    wt0 as 4 per-k pieces (each on its own semaphore - completion counts of
    equal DMAs on a shared semaphore can reorder across queues, which was a
    real intermittent race), issued from SP+ACT in consumption order, so the
    first matmul starts at ~10.6us and the PE never stalls >0.6us after it.
  - margin applied on the diagonal of tile (pair0, rb0): DVE gathers the
    diag, ACT does the trig, DVE rewrites the diagonal; only that one
    unit's out-DMA (issued from gpsimd) gates on the fix.
"""

import math

import numpy as np
import ml_dtypes

import concourse.bass as bass
import concourse.mybir as mybir
from concourse.bass_utils import run_bass_kernel_spmd

AF = mybir.ActivationFunctionType
OP = mybir.AluOpType
F32 = mybir.dt.float32
F16 = mybir.dt.float16
BF16 = mybir.dt.bfloat16

S = 30.0
MARGIN = 0.5
N, D, C = 1024, 512, 100000

NCORES = 8
F = 512               # matmul free dim / class chunk width
F24 = 212             # trimmed width of the leading single chunk
CS = 24 * F + F24     # 12500 classes per core: 8 * 12500 = 100000, no padding
NCHUNK = 25
KD = D // 128         # 4 contraction sub-tiles
NB = N // 128         # 8 row blocks
NWT = 8               # wt chunk buffers
NPB = 8               # pair out-buffer rotation per engine
NPAIR = (NCHUNK - 1) // 2  # 12 pairs after the leading single chunk

COSM = float(math.cos(MARGIN))
SINM = float(math.sin(MARGIN))

# chunk processing order: odd chunk 24 first, then pairs (0,1),(2,3),...
SEQ = [24] + list(range(24))


def _evc(u):
    """('d'|'a', count) such that `sem >= count` implies unit u is evicted.
    DVE evicts even units in order, ACT odd units."""
    if u < 8:
        return ("d" if u % 2 == 0 else "a", u // 2 + 1)
    return ("d" if u % 2 == 0 else "a", 4 + (u - 8) // 2 + 1)


def _munits(oo):
    """s_mmu value once all units of seq-chunk oo are complete."""
    if oo <= 0:
        return NB if oo == 0 else 0
    return NB + NB * ((oo - 1) // 2 + 1)


def build_graph():
    nc = bass.Bass(target_bir_lowering=False)

    eT_ext = nc.declare_dram_parameter("eT", [128, NB * KD * 128], BF16, isOutput=False)
    w_ext = nc.declare_dram_parameter("w", [128, 24 * KD * F + KD * F24], BF16,
                                      isOutput=False)
    ident_ext = nc.declare_dram_parameter("ident", [128, 128], F32, isOutput=False)
    out_ext = nc.declare_dram_parameter("out", [N, CS], F16, isOutput=True)

    import contextlib

    ctx = contextlib.ExitStack()
    sb = lambda name, shape, dt=F32: ctx.enter_context(nc.sbuf_tensor(name, shape, dt))
    sem = lambda name: ctx.enter_context(nc.semaphore(name))

    with ctx:
        # --- SBUF ---
        eT_sb = sb("eT_sb", [128, NB * KD * 128], BF16)
        wt = [sb(f"wt{b}", [128, KD * F], BF16) for b in range(NWT)]
        souts = [sb(f"souts{t}", [128, F24], F16) for t in range(NB)]
        poutd = [sb(f"poutd{b}", [128, 2 * F], F16) for b in range(NPB)]
        pouta = [sb(f"pouta{b}", [128, 2 * F], F16) for b in range(NPB)]
        ident_sb = sb("ident_sb", [128, 128])
        diag_tmp = sb("diag_tmp", [128, 128])
        vdiag = sb("vdiag", [128, 1])
        sqv = sb("sqv", [128, 1])
        lnu = sb("lnu", [128, 1])
        s3v = sb("s3v", [128, 1])
        t1v = sb("t1v", [128, 1])
        fixp = sb("fixp", [128, 1])
        deltap = sb("deltap", [128, 1])
        s2_b = sb("s2_b", [128, 1])

        # --- PSUM: 4 pair-banks [128,1024] = 16KB/partition ---
        ps_pair = [
            ctx.enter_context(nc.psum_tensor(f"ps_pair{b}", [128, 2 * F], F32))
            for b in range(4)
        ]

        # --- semaphores ---
        s_ident = sem("s_ident")
        s_eTt = [sem(f"s_eTt{t}") for t in range(NB)]
        s_ms = sem("s_ms")
        s_do8 = sem("s_do8")
        # one sem per wt0 k-piece: completions of equal-priority DMAs can
        # reorder across queues, so partial counts of a shared sem are unsafe
        s_w24k = [sem(f"s_w24k{k}") for k in range(KD)]
        s_wt = [sem(f"s_wt{b}") for b in range(NWT)]
        s_mmu = sem("s_mmu")       # PE unit done (104 total)
        s_evd = sem("s_evd")       # DVE evictions done
        s_eva = sem("s_eva")       # ACT evictions done
        s_dod = sem("s_dod")       # DVE-side out-DMA completions (x16)
        s_doa = sem("s_doa")       # ACT-side out-DMA completions (x16)
        s_do24 = sem("s_do24")     # chunk-24 out-DMA completions (x16)
        s_vg = sem("s_vg")
        s_sfix = sem("s_sfix")
        s_vfix = sem("s_vfix")
        s_evl = sem("s_evl")       # DVE evicted the last unit's second half

        ev_sem = {"d": s_evd, "a": s_eva}

        with nc.Block() as block:

            @block.gpsimd
            def _(g):
                def wt_dma(o):
                    c = SEQ[o]
                    g.dma_start(
                        out=wt[o % NWT][:],
                        in_=w_ext[:, c * KD * F:(c + 1) * KD * F],
                    ).then_inc(s_wt[o % NWT], 16)

                # gpsimd DMAs go through SWDGE (~6us issue-to-data): only
                # latency-insensitive loads live here; everything the first
                # 20us needs is issued from SP/ACT (HWDGE)
                g.dma_start(out=ident_sb[:], in_=ident_ext[:]).then_inc(s_ident, 16)
                g.memset(s2_b[:], float(S * S)).then_inc(s_ms, 1)
                # unit 8 (pair0, rb0) holds the label diagonal: its out-DMA
                # is the only one gated on the margin fix, issued from here
                # (the evicting engines issue every other out-DMA themselves)
                g.wait_ge(s_vfix, 1)
                g.dma_start(
                    out=out_ext[0:128, 0:2 * F], in_=poutd[0][:],
                ).then_inc(s_do8, 16)
                # wt5/wt6 deferred past the critical eT/wt0/wt1/wt2 window:
                # the margin fix (~pair0 mid) is long before pair 2 needs them
                wt_dma(5)
                wt_dma(6)
                for p in range(NPAIR):
                    for o in (2 * p + 7, 2 * p + 8):
                        if o <= NCHUNK - 1:
                            oo = o - NWT
                            if oo >= 0:
                                g.wait_ge(s_mmu, _munits(oo))
                            wt_dma(o)
                g.wait_ge(s_do24, 16 * NB)
                g.wait_ge(s_do8, 16)
                g.wait_ge(s_dod, 16 * (4 * NPAIR - 1))
                g.wait_ge(s_doa, 16 * 4 * NPAIR)

            @block.scalar
            def _(s):
                # dummy op: pulls the ACT table load off the critical path
                s.activation(sqv[:], vdiag[:], AF.Square)
                # ACT issues the wt0 k-pieces (tiny, needed first), the odd
                # eT pieces, then the pair-0 chunks, in consumption order
                W24 = 24 * KD * F
                for k in range(KD):
                    s.dma_start(
                        out=wt[0][:, k * F24:(k + 1) * F24],
                        in_=w_ext[:, W24 + k * F24:W24 + (k + 1) * F24],
                    ).then_inc(s_w24k[k], 16)
                for t_ in (1, 3, 5, 7):
                    s.dma_start(
                        out=eT_sb[:, t_ * KD * 128:(t_ + 1) * KD * 128],
                        in_=eT_ext[:, t_ * KD * 128:(t_ + 1) * KD * 128],
                    ).then_inc(s_eTt[t_], 16)
                s.dma_start(
                    out=wt[1][:],
                    in_=w_ext[:, SEQ[1] * KD * F:(SEQ[1] + 1) * KD * F],
                ).then_inc(s_wt[1], 16)
                s.dma_start(
                    out=wt[2][:],
                    in_=w_ext[:, SEQ[2] * KD * F:(SEQ[2] + 1) * KD * F],
                ).then_inc(s_wt[2], 16)
                # chunk-24 odd tiles; the out-DMA is issued right here on the
                # scalar sequencer (in-order after the eviction, HWDGE)
                for tt in (1, 3, 5, 7):
                    s.wait_ge(s_mmu, tt + 1)
                    s.activation(
                        souts[tt][:],
                        ps_pair[tt % 4][:, (tt // 4) * F:(tt // 4) * F + F24],
                        AF.Copy,
                    ).then_inc(s_eva, 1)
                    s.dma_start(
                        out=out_ext[tt * 128:(tt + 1) * 128, 24 * F:24 * F + F24],
                        in_=souts[tt][:],
                    ).then_inc(s_do24, 16)
                    if tt == 5:
                        s.dma_start(
                            out=wt[3][:],
                            in_=w_ext[:, SEQ[3] * KD * F:(SEQ[3] + 1) * KD * F],
                        ).then_inc(s_wt[3], 16)
                    if tt == 7:
                        s.dma_start(
                            out=wt[4][:],
                            in_=w_ext[:, SEQ[4] * KD * F:(SEQ[4] + 1) * KD * F],
                        ).then_inc(s_wt[4], 16)
                # odd pair units
                for j in range(4 * NPAIR):
                    u = 9 + 2 * j
                    p, nb = (u - 8) // 8, (u - 8) % 8
                    if j >= NPB:
                        s.wait_ge(s_doa, 16 * (j - NPB + 1))
                    s.wait_ge(s_mmu, u + 1)
                    if j == 4 * NPAIR - 1:
                        # final unit: evict/DMA only the first chunk here;
                        # DVE+SP handle the second half in parallel to
                        # shorten the kernel tail
                        s.activation(pouta[j % NPB][:, 0:F],
                                     ps_pair[u % 4][:, 0:F],
                                     AF.Copy).then_inc(s_eva, 1)
                        s.dma_start(
                            out=out_ext[nb * 128:(nb + 1) * 128,
                                        2 * p * F:(2 * p + 1) * F],
                            in_=pouta[j % NPB][:, 0:F],
                        ).then_inc(s_doa, 16)
                        continue
                    s.activation(pouta[j % NPB][:], ps_pair[u % 4][:],
                                 AF.Copy).then_inc(s_eva, 1)
                    s.dma_start(
                        out=out_ext[nb * 128:(nb + 1) * 128,
                                    2 * p * F:(2 * p + 2) * F],
                        in_=pouta[j % NPB][:],
                    ).then_inc(s_doa, 16)
                    if j == 0:
                        # margin trig on the gathered diagonal [128,1]
                        s.wait_ge(s_vg, 1)
                        s.wait_ge(s_ms, 1)
                        s.activation(sqv[:], vdiag[:], AF.Square)
                        s.drain()
                        s.activation(lnu[:], sqv[:], AF.Ln, scale=-1.0, bias=s2_b[:])
                        s.drain()
                        s.activation(s3v[:], lnu[:], AF.Exp, scale=0.5)
                        s.activation(t1v[:], vdiag[:], AF.Copy, scale=COSM)
                        s.drain().then_inc(s_sfix, 1)

            @block.tensor
            def _(t):
                # chunk 24 (trimmed to F24): tile tt -> bank tt%4, half tt//4
                for tt in range(NB):
                    t.wait_ge(s_eTt[tt], 16)
                    bank, half = tt % 4, tt // 4
                    for k in range(KD):
                        if tt == 0:
                            t.wait_ge(s_w24k[k], 16)
                        mm = t.matmul(
                            ps_pair[bank][:, half * F:half * F + F24],
                            lhsT=eT_sb[:, tt * 512 + k * 128:tt * 512 + (k + 1) * 128],
                            rhs=wt[0][:, k * F24:(k + 1) * F24],
                            start=(k == 0), stop=(k == KD - 1),
                            skip_group_check=True,
                        )
                    mm.then_inc(s_mmu, 1)

                def wt_thr(o):
                    # wt[0]'s initial fill is tracked on s_w24k instead, so
                    # s_wt[0] only counts its refills (o = 8, 16, 24)
                    if o % NWT == 0:
                        return 16 * (o // NWT)
                    return 16 * (o // NWT + 1)

                for p in range(NPAIR):
                    o0, o1 = 2 * p + 1, 2 * p + 2
                    t.wait_ge(s_wt[o0 % NWT], wt_thr(o0))
                    t.wait_ge(s_wt[o1 % NWT], wt_thr(o1))
                    for nb in range(NB):
                        u = NB + NB * p + nb
                        # bank u%4 free once its previous occupants evicted
                        if u < 12:
                            b = u - 8
                            sd, cnt = _evc(b)          # tile b (half 0)
                            _, cnt2 = _evc(b + 4)      # tile b+4 (half 1)
                            t.wait_ge(ev_sem[sd], max(cnt, cnt2))
                        else:
                            sd, cnt = _evc(u - 4)
                            t.wait_ge(ev_sem[sd], cnt)
                        for k in range(KD):
                            for j, ob in ((0, o0), (1, o1)):
                                mm = t.matmul(
                                    ps_pair[u % 4][:, j * F:(j + 1) * F],
                                    lhsT=eT_sb[:, nb * 512 + k * 128:
                                               nb * 512 + (k + 1) * 128],
                                    rhs=wt[ob % NWT][:, k * F:(k + 1) * F],
                                    start=(k == 0), stop=(k == KD - 1),
                                    skip_group_check=True,
                                )
                        mm.then_inc(s_mmu, 1)

            @block.sync
            def _(sy):
                # SP issues the even eT pieces, eT0 first so tile 0 can
                # start as early as possible
                for t_ in (0, 2, 4, 6):
                    sy.dma_start(
                        out=eT_sb[:, t_ * KD * 128:(t_ + 1) * KD * 128],
                        in_=eT_ext[:, t_ * KD * 128:(t_ + 1) * KD * 128],
                    ).then_inc(s_eTt[t_], 16)
                # DVE can't drive HWDGE; the otherwise-idle SP sequencer
                # issues the out-DMAs for DVE's evictions, paced by s_evd
                for tt in (0, 2, 4, 6):
                    sy.wait_ge(s_evd, tt // 2 + 1)
                    sy.dma_start(
                        out=out_ext[tt * 128:(tt + 1) * 128, 24 * F:24 * F + F24],
                        in_=souts[tt][:],
                    ).then_inc(s_do24, 16)
                # unit 8 (j==0) is DMAed from gpsimd after the margin fix
                for j in range(1, 4 * NPAIR):
                    u = 8 + 2 * j
                    p, nb = (u - 8) // 8, (u - 8) % 8
                    sy.wait_ge(s_evd, 4 + j + 1)
                    sy.dma_start(
                        out=out_ext[nb * 128:(nb + 1) * 128,
                                    2 * p * F:(2 * p + 2) * F],
                        in_=poutd[j % NPB][:],
                    ).then_inc(s_dod, 16)
                # second half of unit 103, evicted by DVE in parallel with
                # ACT's first half
                sy.wait_ge(s_evl, 1)
                sy.dma_start(
                    out=out_ext[7 * 128:8 * 128, 23 * F:24 * F],
                    in_=pouta[7][:, F:2 * F],
                ).then_inc(s_doa, 16)

            @block.vector
            def _(v):
                # chunk-24 even tiles
                for tt in (0, 2, 4, 6):
                    v.wait_ge(s_mmu, tt + 1)
                    v.tensor_scalar_mul(
                        souts[tt][:],
                        ps_pair[tt % 4][:, (tt // 4) * F:(tt // 4) * F + F24],
                        1.0,
                    ).then_inc(s_evd, 1)
                # even pair units
                for j in range(4 * NPAIR):
                    u = 8 + 2 * j
                    if j == NPB:
                        # poutd[0] was DMAed by gpsimd (vfix path), tracked
                        # on its own semaphore
                        v.wait_ge(s_do8, 16)
                    elif j > NPB:
                        v.wait_ge(s_dod, 16 * (j - NPB))
                    v.wait_ge(s_mmu, u + 1)
                    v.tensor_scalar_mul(poutd[j % NPB][:], ps_pair[u % 4][:],
                                        1.0).then_inc(s_evd, 1)
                    if j == 0:
                        # extract the label diagonal of tile (pair0, rb0)
                        v.wait_ge(s_ident, 16)
                        v.drain()
                        v.tensor_tensor(out=diag_tmp[:], in0=poutd[0][:, 0:128],
                                        in1=ident_sb[:], op=OP.mult)
                        v.drain()
                        v.tensor_reduce(vdiag[:], diag_tmp[:],
                                        mybir.AxisListType.X, OP.add)
                        v.drain().then_inc(s_vg, 1)
                    if j == 4 * NPAIR - 1:
                        # second half of the final (ACT) unit 103: evict it
                        # here in parallel with ACT's first-half evict
                        v.wait_ge(s_doa, 16 * 40)  # pouta[7] free (j=39 DMAed)
                        v.wait_ge(s_mmu, 104)
                        v.tensor_scalar_mul(pouta[7][:, F:2 * F],
                                            ps_pair[3][:, F:2 * F],
                                            1.0).then_inc(s_evl, 1)
                    if j == 2:
                        # margin rewrite of the diagonal after the trig lands
                        v.wait_ge(s_sfix, 1)
                        v.scalar_tensor_tensor(fixp[:], s3v[:], -SINM, t1v[:],
                                               OP.mult, OP.add)
                        v.drain()
                        v.tensor_tensor(out=deltap[:], in0=fixp[:], in1=vdiag[:],
                                        op=OP.subtract)
                        v.drain()
                        v.scalar_tensor_tensor(poutd[0][:, 0:128], ident_sb[:],
                                               deltap[:], poutd[0][:, 0:128],
                                               OP.mult, OP.add)
                        v.drain().then_inc(s_vfix, 1)

    return nc


_GRAPH = None


def _get_graph():
    global _GRAPH
    if _GRAPH is None:
        _GRAPH = build_graph()
    return _GRAPH


def _host_prepare(embeddings, weight, labels):
    """Row/class permutations putting each core's labels on the (0,0) diagonal,
    with both L2 norms and the S scale folded in, packed into
    partition-contiguous bf16 layouts.

    CS * NCORES == C exactly, so every class gets exactly one column.  When a
    class is the label of two rows, the later row's diagonal slot is given a
    replacement (unlabeled) class instead; both affected entries of the output
    are recomputed exactly on the host afterwards (a handful per batch)."""
    labels = np.asarray(labels).astype(np.int64)
    e = np.asarray(embeddings, dtype=np.float32)
    w = np.asarray(weight, dtype=np.float32)

    # host-side normalization (matches torch F.normalize eps semantics)
    en = S * e / np.maximum(np.linalg.norm(e, axis=1, keepdims=True), 1e-12)
    wn = w / np.maximum(np.linalg.norm(w, axis=1, keepdims=True), 1e-12)

    seen = set()
    dups = []  # (row i, label) for duplicate label classes
    for i in range(N):
        l = int(labels[i])
        if l in seen:
            dups.append((i, l))
        else:
            seen.add(l)

    labeled = np.zeros(C, dtype=bool)
    labeled[labels] = True
    unlab = list(np.nonzero(~labeled)[0])

    colmaps = np.empty((NCORES, CS), dtype=np.int64)
    for i in range(N):
        colmaps[i // 128, i % 128] = labels[i]
    fixes = []  # (row i, label l, replacement r)
    for (i, l) in dups:
        r = int(unlab.pop())
        colmaps[i // 128, i % 128] = r
        fixes.append((i, l, r))
    colmaps[:, 128:] = np.asarray(unlab, dtype=np.int64).reshape(
        NCORES, CS - 128)

    e_bf = en.astype(ml_dtypes.bfloat16)
    wTfull = wn.T  # [512, 100000] view
    ident = np.eye(128, dtype=np.float32)
    in_maps = []
    row_perms = []
    for m in range(NCORES):
        wsel = np.ascontiguousarray(wTfull[:, colmaps[m]])
        w_bf = wsel.astype(ml_dtypes.bfloat16)
        # [D=(ko,p), c] -> [p, chunk, ko, f] for the 24 full chunks,
        # then [p, ko, f24] for the trimmed chunk
        wmain = np.ascontiguousarray(
            w_bf[:, :24 * F].reshape(KD, 128, 24, F).transpose(1, 2, 0, 3)
        ).reshape(128, 24 * KD * F)
        wtail = np.ascontiguousarray(
            w_bf[:, 24 * F:].reshape(KD, 128, F24).transpose(1, 0, 2)
        ).reshape(128, KD * F24)
        wprep = np.concatenate([wmain, wtail], axis=1)
        rows = np.concatenate([
            np.arange(m * 128, (m + 1) * 128),
            np.delete(np.arange(N), np.s_[m * 128:(m + 1) * 128]),
        ])
        row_perms.append(rows)
        e_perm = e_bf[rows]                      # [N, D]
        # [D=(k,dp), N=(nb,np)] -> [dp, nb, k, np]
        eTprep = np.ascontiguousarray(
            e_perm.T.reshape(KD, 128, NB, 128).transpose(1, 2, 0, 3)
        ).reshape(128, NB * KD * 128)
        in_maps.append({
            "eT": eTprep,
            "w": wprep,
            "ident": ident,
        })
    return in_maps, row_perms, colmaps, fixes, en, wn


def _assemble(results, row_perms, colmaps, fixes, en, wn):
    out = np.empty((N, C), dtype=np.float32)
    for m in range(NCORES):
        slab = results[m]["out"].astype(np.float32)
        unperm = np.empty_like(slab)
        unperm[row_perms[m]] = slab
        out[:, colmaps[m]] = unperm
    for (i, l, r) in fixes:
        # r's column got the margin on the device at row i: recompute exactly
        out[i, r] = float(en[i] @ wn[r])
        # row i's true label column is elsewhere, unmargined: apply margin
        cos = min(1.0, max(-1.0, out[i, l] / S))
        out[i, l] = S * math.cos(math.acos(cos) + MARGIN)
    return out


def kernel(embeddings, weight, labels, _trace=False):
    nc = _get_graph()
    in_maps, row_perms, colmaps, fixes, en, wn = _host_prepare(
        embeddings, weight, labels
    )
    res = run_bass_kernel_spmd(nc, in_maps, core_ids=list(range(NCORES)), trace=_trace)
    out = _assemble(res.results, row_perms, colmaps, fixes, en, wn)
    if _trace:
        return out, res
    return out
